# revision 3
# baseline (speedup 1.0000x reference)
"""BiGraphConv v2: batched DMA-gather + bf16 SpMM on 8 Trainium2 cores.

Differences from the v1 baseline (per-chunk indirect DMA, f32):
  - Edges gathered with InstDMAGatherAnt: thousands of rows per
    instruction instead of 128, killing the per-instruction SWDGE
    descriptor-generation overhead that dominated v1.
  - b_input is converted to bf16 on the host; gathers move 256B rows
    (half the HBM traffic), and the SpMM matmuls run at the 1 cyc/row
    bf16 rate instead of 4 cyc/row f32.
  - Output blocks of 256 rows (one-hot scatter width 256) halve the
    number of PSUM accumulation groups.
  - dma_gather indices are int16, so the 100k b-rows are split into 4
    col-groups of 25k; each block's edges are binned by (block, group)
    on the host and padded to 128-edge chunks.

Per core d (owns output rows [d*12500, (d+1)*12500)):
  AllGather replicates bf16 b (3.2MB shard -> 25.6MB full).
  For each slab (run of blocks whose chunks fit the gather windows):
    4 dma_gather instructions (one per col-group) fill bf16 windows
    [128, W_CH, 128] with gathered rows, double buffered.
    Per block: accumulate chunk matmuls into PSUM y2[f=128, r=256]
    (lhsT = gathered chunk [e,f], rhs = one-hot scatter s_t [e,r] built
    on DVE by iota==rr * vv), then y2 -> bf16 SBUF, two [128,128]
    matmuls against W give out[r, f] directly, + bias, DMA out.

kernel(**inputs) takes FULL inputs, returns FULL [100000,128] f32.
Self-contained: shapes/sharding hardcoded.
"""

import numpy as np

import concourse.bass as bass
import concourse.bacc as bacc
import concourse.mybir as mybir
import concourse.tile as tile
from concourse.bass_utils import run_bass_kernel_spmd
from concourse.library_config import mlp as _mlp_lib

import ml_dtypes

NA = 100000
NB = 100000
NE = 1600000
F = 128
N_CORES = 8
ROWS_PER_CORE = NA // N_CORES          # 12500
RW = 256                               # output rows per block
NBLK = -(-ROWS_PER_CORE // RW)         # 49 blocks per core
OUT_ROWS = NBLK * RW                   # 12544 (padded, host slices)
G = 4                                  # col groups (int16 index range)
GROUP_ROWS = NB // G                   # 25000
CHUNK = 128                            # edges per matmul
W_CH = 48        # gather window capacity in chunks
NO_COLLECTIVE = False
ONE_QUEUE = False
NO_LOADLIB = False
MAX_SLABS = 0
GCAP = 8         # max chunks (128 idxs) per dma_gather (HW caps ~1024-2048 idxs)

BF16 = ml_dtypes.bfloat16

LAST_RESULTS = None
LAST_SPMD_WALL_NS = None


def _host_prep(edge_rows, edge_cols, edge_vals):
    """Sort edges by (core, block, group); build slot arrays + schedule.

    SPMD runs one module on all 8 cores, so the schedule (slab layout,
    chunk counts) is shared: per-(block,group) chunk counts are maxed
    across cores.  Each core fills its own idx/rr/vv slot arrays; slack
    slots gather row 0 with weight 0.

    Returns (sched, per_core) where sched has slabs/instrs/blk_sched/
    tot_chunks and per_core has the upload arrays.
    """
    rows = np.asarray(edge_rows)
    cols = np.asarray(edge_cols)
    vals = np.asarray(edge_vals)

    order = np.argsort(rows, kind="stable")
    rows = rows[order]
    cols = cols[order]
    vals = vals[order]
    core_bounds = np.searchsorted(rows, np.arange(N_CORES + 1) * ROWS_PER_CORE)

    raw = []
    cnts = np.zeros((N_CORES, NBLK * G), dtype=np.int64)
    for d in range(N_CORES):
        a, b = core_bounds[d], core_bounds[d + 1]
        r = rows[a:b] - d * ROWS_PER_CORE
        c = cols[a:b]
        v = vals[a:b]
        key = (r >> 8) * G + c // GROUP_ROWS
        o2 = np.argsort(key, kind="stable")
        r, c, v, key = r[o2], c[o2], v[o2], key[o2]
        cnts[d] = np.bincount(key, minlength=NBLK * G)
        raw.append((r, c, v, key))

    chunks = -(-cnts.max(axis=0) // CHUNK)         # ceil of per-key max
    chunks = np.maximum(chunks, 1)                 # every (blk,g) present

    # greedy slabs: consecutive blocks; per-group chunk sum <= W_CH
    slabs = []
    cur = [0]
    gsum = chunks[0:G].astype(np.int64).copy()
    for bb in range(1, NBLK):
        nxt = gsum + chunks[bb * G:(bb + 1) * G]
        if (nxt <= W_CH).all():
            cur.append(bb)
            gsum = nxt
        else:
            slabs.append(cur)
            cur = [bb]
            gsum = chunks[bb * G:(bb + 1) * G].astype(np.int64).copy()
    slabs.append(cur)

    # slot layout: for slab: for g: for blk in slab: chunks
    key_chunk_start = np.zeros(NBLK * G, dtype=np.int64)
    instrs = []              # per slab: list of (g, chunk_start, n_chunks)
    blk_sched_map = {}       # blk -> list of (g, wstart, nch, instr_chunk_start)
    pos = 0
    for slab in slabs:
        sl_instrs = []
        for g in range(G):
            istart = pos
            for bb in slab:
                k = bb * G + g
                key_chunk_start[k] = pos
                blk_sched_map.setdefault(bb, []).append(
                    (g, pos - istart, int(chunks[k]), istart))
                pos += int(chunks[k])
            sl_instrs.append((g, istart, pos - istart))
        instrs.append(sl_instrs)
    tot_chunks = pos
    tot_slots = tot_chunks * CHUNK

    sched = {
        "slabs": slabs, "instrs": instrs,
        "blk_sched": blk_sched_map, "tot_chunks": tot_chunks,
    }

    per_core = []
    for d in range(N_CORES):
        r, c, v, key = raw[d]
        cnt = cnts[d]
        gstart = np.zeros(NBLK * G + 1, dtype=np.int64)
        np.cumsum(cnt, out=gstart[1:])
        rank = np.arange(len(r)) - gstart[key]
        slot = key_chunk_start[key] * CHUNK + rank

        idx_arr = np.zeros(tot_slots, dtype=np.int16)
        rr_arr = np.zeros(tot_slots, dtype=np.float32)
        vv_arr = np.zeros(tot_slots, dtype=np.float32)
        idx_arr[slot] = (c % GROUP_ROWS).astype(np.int16)
        rr_arr[slot] = (r & (RW - 1)).astype(np.float32)
        vv_arr[slot] = v

        # dma_gather idx layout: [128, tot_slots//16] with
        # [p, w] = idx_arr[w*16 + p%16]  (16-wrapped, replicated x8)
        idx16 = idx_arr.reshape(-1, 16).T            # [16, tot_slots//16]
        idx_l = np.ascontiguousarray(np.tile(idx16, (8, 1)))

        rr_l = np.ascontiguousarray(rr_arr.reshape(tot_chunks, CHUNK).T)
        vv_l = np.ascontiguousarray(vv_arr.reshape(tot_chunks, CHUNK).T)

        per_core.append({"idx": idx_l, "rr": rr_l, "vv": vv_l})
    return sched, per_core


def _split_waits(nc, max_waits=1):
    """Walrus CTRL ops encode one sem wait; peel extras onto chained drains."""
    for fn in nc.m.functions:
        for bb in fn.blocks:
            new_insts = []
            for inst in bb.instructions:
                si = inst.sync_info
                if si is not None and si.on_wait and len(si.on_wait) > max_waits:
                    waits = list(si.on_wait)
                    while len(waits) > max_waits:
                        chunk, waits = waits[:max_waits], waits[max_waits:]
                        d = mybir.InstDrain(
                            name=nc.get_next_instruction_name(),
                            ins=[], outs=[], bass_is_fusable=False,
                        )
                        d.engine = inst.engine
                        d.sync_info = mybir.SyncInfo(on_wait=chunk, on_update=[])
                        nc.register_instruction(d)
                        new_insts.append(d)
                    si.on_wait = waits
                new_insts.append(inst)
            bb.instructions[:] = new_insts


def _build(sched):
    """Build the (shared, SPMD) Bass module from the unified schedule."""
    f32 = mybir.dt.float32
    bf16 = mybir.dt.bfloat16
    i16 = mybir.dt.int16

    tot_chunks = sched["tot_chunks"]
    tot_slots = tot_chunks * CHUNK
    instrs = sched["instrs"]
    slabs = sched["slabs"]
    blk_sched = sched["blk_sched"]

    nc = bacc.Bacc("TRN2", target_bir_lowering=False, num_swdge_queues=4)
    if NO_COLLECTIVE:
        b_full = nc.declare_dram_parameter("b_full", [NB, F], bf16, isOutput=False)
    else:
        b_shard = nc.declare_dram_parameter(
            "b_shard", [NB // N_CORES, F], bf16, isOutput=False)
        b_shard_int = nc.dram_tensor("b_shard_int", [NB // N_CORES, F], bf16)
        b_full = nc.dram_tensor("b_full", [NB, F], bf16, addr_space="Shared")
    w_d = nc.declare_dram_parameter("w", [F, F], bf16, isOutput=False)
    bias_d = nc.declare_dram_parameter("bias_bcast", [128, F], f32, isOutput=False)
    iota_d = nc.declare_dram_parameter("iota", [128, RW], bf16, isOutput=False)
    idx_d = nc.declare_dram_parameter(
        "idx", [128, tot_slots // 16], i16, isOutput=False)
    rr_d = nc.declare_dram_parameter("rr", [128, tot_chunks], f32, isOutput=False)
    vv_d = nc.declare_dram_parameter("vv", [128, tot_chunks], f32, isOutput=False)
    out_d = nc.declare_dram_parameter("out", [OUT_ROWS, F], f32, isOutput=True)

    with tile.TileContext(nc) as tc:
        with (
            tc.tile_pool(name="const", bufs=1) as const_pool,
            tc.tile_pool(name="meta", bufs=1) as meta_pool,
            tc.tile_pool(name="gather", bufs=2) as gather_pool,
            tc.tile_pool(name="st", bufs=8) as st_pool,
            tc.tile_pool(name="y2sb", bufs=3) as y2sb_pool,
            tc.tile_pool(name="outsb", bufs=4) as outsb_pool,
            tc.tile_pool(name="y2ps", bufs=2, space="PSUM") as y2ps_pool,
            tc.tile_pool(name="outps", bufs=2, space="PSUM") as outps_pool,
        ):
            w_sb = const_pool.tile([F, F], bf16)
            bias_sb = const_pool.tile([128, F], f32)
            iota_sb = const_pool.tile([128, RW], bf16)
            nc.sync.dma_start(out=w_sb[:], in_=w_d[:])
            nc.sync.dma_start(out=bias_sb[:], in_=bias_d[:])
            nc.sync.dma_start(out=iota_sb[:], in_=iota_d[:])

            idx_sb = meta_pool.tile([128, tot_slots // 16], i16)
            rr_sb = meta_pool.tile([128, tot_chunks], f32)
            vv_sb = meta_pool.tile([128, tot_chunks], f32)
            nc.sync.dma_start(out=idx_sb[:], in_=idx_d[:])
            nc.sync.dma_start(out=rr_sb[:], in_=rr_d[:])
            nc.sync.dma_start(out=vv_sb[:], in_=vv_d[:])

            if not NO_LOADLIB:
                nc.gpsimd.load_library(_mlp_lib)
            if not NO_COLLECTIVE:
                nc.sync.dma_start(out=b_shard_int[:], in_=b_shard[:])
                nc.gpsimd.collective_compute(
                    "AllGather",
                    mybir.AluOpType.bypass,
                    replica_groups=[list(range(N_CORES))],
                    ins=[b_shard_int[:]],
                    outs=[b_full[:]],
                )

            for s, slab in enumerate(slabs):
                if MAX_SLABS and s >= MAX_SLABS:
                    break
                gt = {}
                for (g, istart, nch) in instrs[s]:
                    if nch == 0:
                        continue
                    t = gather_pool.tile([128, W_CH, F], bf16, tag=f"g{g}")
                    gt[g] = (t, istart)
                    for o in range(0, nch, GCAP):
                        cnt = min(GCAP, nch - o)
                        n_idxs = cnt * CHUNK
                        nc.gpsimd.dma_gather(
                            out_ap=t[:, o:o + cnt, :],
                            in_ap=b_full[g * GROUP_ROWS:(g + 1) * GROUP_ROWS, :],
                            idxs_ap=idx_sb[:, (istart + o) * 8:(istart + o + cnt) * 8],
                            num_idxs=n_idxs,
                            num_idxs_reg=n_idxs,
                            elem_size=F,
                            queue_num=0 if ONE_QUEUE else g % 4,
                        )
                for bb in slab:
                    sched = blk_sched[bb]
                    tot_mm = sum(nch for (_, _, nch, _) in sched)
                    y2 = y2ps_pool.tile([F, RW], f32, tag="y2")
                    mm = 0
                    for (g, wstart, nch, istart) in sched:
                        t, _ = gt[g]
                        for k in range(nch):
                            gchunk = istart + wstart + k
                            s_t = st_pool.tile([128, RW], bf16, tag="s_t")
                            nc.vector.tensor_scalar(
                                out=s_t[:],
                                in0=iota_sb[:],
                                scalar1=rr_sb[:, gchunk:gchunk + 1],
                                scalar2=vv_sb[:, gchunk:gchunk + 1],
                                op0=mybir.AluOpType.is_equal,
                                op1=mybir.AluOpType.mult,
                            )
                            nc.tensor.matmul(
                                out=y2[:],
                                lhsT=t[:, wstart + k, :],
                                rhs=s_t[:],
                                start=(mm == 0),
                                stop=(mm == tot_mm - 1),
                            )
                            mm += 1
                    y2_sb = y2sb_pool.tile([F, RW], bf16, tag="y2sb")
                    nc.scalar.activation(
                        out=y2_sb[:], in_=y2[:],
                        func=mybir.ActivationFunctionType.Copy,
                    )
                    for h in range(RW // 128):
                        o_ps = outps_pool.tile([128, F], f32, tag="ops")
                        nc.tensor.matmul(
                            out=o_ps[:],
                            lhsT=y2_sb[:, h * 128:(h + 1) * 128],
                            rhs=w_sb[:],
                            start=True, stop=True,
                        )
                        o_sb = outsb_pool.tile([128, F], f32, tag="osb")
                        nc.vector.tensor_tensor(
                            out=o_sb[:], in0=o_ps[:], in1=bias_sb[:],
                            op=mybir.AluOpType.add,
                        )
                        nc.sync.dma_start(
                            out=out_d[bb * RW + h * 128: bb * RW + (h + 1) * 128, :],
                            in_=o_sb[:],
                        )
    nc.finalize()
    return nc


def prepare(b_input, edge_rows, edge_cols, edge_vals, a_weight, a_bias):
    b_input = np.ascontiguousarray(np.asarray(b_input, dtype=np.float32))
    a_weight = np.ascontiguousarray(np.asarray(a_weight, dtype=np.float32))
    a_bias = np.asarray(a_bias, dtype=np.float32)

    sched, per_core = _host_prep(edge_rows, edge_cols, edge_vals)
    nc = _build(sched)

    bias_bcast = np.tile(a_bias[None, :], (128, 1)).astype(np.float32)
    iota = np.tile(np.arange(RW, dtype=np.float32)[None, :], (128, 1)).astype(BF16)
    w_bf = a_weight.astype(BF16)
    b_bf = b_input.astype(BF16)

    in_maps = []
    for d in range(N_CORES):
        bkey = ("b_full", b_bf) if NO_COLLECTIVE else (
            "b_shard", b_bf[d * (NB // N_CORES):(d + 1) * (NB // N_CORES)])
        in_maps.append({
            bkey[0]: bkey[1],
            "w": w_bf,
            "bias_bcast": bias_bcast,
            "iota": iota,
            "idx": per_core[d]["idx"],
            "rr": per_core[d]["rr"],
            "vv": per_core[d]["vv"],
        })

    def post(results):
        out = np.empty((NA, F), dtype=np.float32)
        for d in range(N_CORES):
            out[d * ROWS_PER_CORE:(d + 1) * ROWS_PER_CORE] = (
                results[d]["out"][:ROWS_PER_CORE]
            )
        return out

    return nc, in_maps, post


def kernel(b_input, edge_rows, edge_cols, edge_vals, a_weight, a_bias):
    global LAST_RESULTS, LAST_SPMD_WALL_NS
    nc, in_maps, post = prepare(
        b_input, edge_rows, edge_cols, edge_vals, a_weight, a_bias)

    import time as _time
    _t0 = _time.time()
    res = run_bass_kernel_spmd(nc, in_maps, core_ids=list(range(N_CORES)))
    LAST_SPMD_WALL_NS = int((_time.time() - _t0) * 1e9)
    LAST_RESULTS = res
    return post(res.results)
